# revision 8
# baseline (speedup 1.0000x reference)
"""AttentionAggregation kernel for 8 TRN2 NeuronCores.

Math: out[b] = mean_n softmax(Q K^T)[n,:] @ V  with Q/K/V = x @ W^T + b.
Key algebraic fold: out[b,d] = sum_m w[b,m] V[b,m,d] with
  w[b,m] = (1/N) sum_n exp(S[n,m]) / R[n],  R[n] = sum_m exp(S[n,m]).
So the attn@V matmul (N^2 D work) collapses to a column-weight vector w
computed with rank-1 matmuls (r^T @ E), then a single weighted reduction
against V. Softmax max-subtraction is skipped: |S| <= ~25 here, exp stays
comfortably inside fp32 range and softmax is shift-invariant.

Sharding: core c handles batch b=c//2, row half h=c%2 (2048 rows of the
4096-row softmax). Host sums the two per-core partial outputs per batch.
"""

import sys

sys.path.insert(0, "/opt/trn_rl_repo")

import numpy as np

import concourse.bass as bass
import concourse.mybir as mybir
import concourse.tile as tile
from concourse import bacc

D = 128
N = 4096
B = 4
NCORES = 8
HALF = N // 2  # softmax rows per core
RT = HALF // 128  # 16 row tiles per core
GW = 1024  # psum group width (2 banks) per exp instruction
NG = N // GW  # 4 exp groups per row tile
NCH = N // 512  # 8 m-chunks of 512

F32 = mybir.dt.float32
AF = mybir.ActivationFunctionType
ALU = mybir.AluOpType


def build_nc():
    nc = bacc.Bacc()
    xt = nc.dram_tensor("xt", [D, N], F32, kind="ExternalInput")  # x[b].T
    xqt = nc.dram_tensor("xqt", [D, HALF], F32, kind="ExternalInput")  # row-half of x[b].T
    wqT = nc.dram_tensor("wqT", [D, D], F32, kind="ExternalInput")  # Wq.T
    wkT = nc.dram_tensor("wkT", [D, D], F32, kind="ExternalInput")
    wvT = nc.dram_tensor("wvT", [D, D], F32, kind="ExternalInput")
    bq = nc.dram_tensor("bq", [D, 1], F32, kind="ExternalInput")
    bk = nc.dram_tensor("bk", [D, 1], F32, kind="ExternalInput")
    bv = nc.dram_tensor("bv", [D, 1], F32, kind="ExternalInput")
    out = nc.dram_tensor("out", [D, 1], F32, kind="ExternalOutput")

    with tile.TileContext(nc) as tc:
        with (
            tc.tile_pool(name="singles", bufs=1) as singles,
            tc.tile_pool(name="sp", bufs=3, space="PSUM") as sp,
            tc.tile_pool(name="wp", bufs=1, space="PSUM") as wp,
            tc.tile_pool(name="epool", bufs=2) as epool,
            tc.tile_pool(name="small", bufs=3) as small,
        ):
            # ---- constants / weights ----
            wq_sb = singles.tile([D, D], F32, tag="wq", name="wq_sb")
            wk_sb = singles.tile([D, D], F32, tag="wk", name="wk_sb")
            wv_sb = singles.tile([D, D], F32, tag="wv", name="wv_sb")
            nc.sync.dma_start(wk_sb, wkT[:, :])
            nc.sync.dma_start(wq_sb, wqT[:, :])
            nc.sync.dma_start(wv_sb, wvT[:, :])
            bqs = singles.tile([D, 1], F32, tag="bq", name="bqs")
            bks = singles.tile([D, 1], F32, tag="bk", name="bks")
            bvs = singles.tile([D, 1], F32, tag="bv", name="bvs")
            nc.sync.dma_start(bks, bk[:, :])
            nc.sync.dma_start(bqs, bq[:, :])
            nc.sync.dma_start(bvs, bv[:, :])
            ones_sb = singles.tile([D, D], F32, tag="ones", name="ones_sb")
            nc.vector.memset(ones_sb, 1.0)

            # ---- activations in ----
            xt_sb = singles.tile([D, N], F32, tag="xt", name="xt_sb")
            for c in range(NCH):
                nc.sync.dma_start(xt_sb[:, c * 512 : (c + 1) * 512], xt[:, c * 512 : (c + 1) * 512])
            xq_sb = singles.tile([D, HALF], F32, tag="xq", name="xq_sb")
            for c in range(HALF // 512):
                nc.sync.dma_start(xq_sb[:, c * 512 : (c + 1) * 512], xqt[:, c * 512 : (c + 1) * 512])

            kt_sb = singles.tile([D, N], F32, tag="kt", name="kt_sb")
            qt_sb = singles.tile([D, HALF], F32, tag="qt", name="qt_sb")
            vt_sb = singles.tile([D, N], F32, tag="vt", name="vt_sb")

            # ---- projections: out = W^T.T @ x^T (+bias), landing transposed [e, n] ----
            def proj(dst, w_sb, src_sb, bias_sb, width, pfx):
                for g in range(width // GW):
                    pt = sp.tile([128, GW], F32, tag="spg", name=f"{pfx}_{g}")
                    for hh in range(2):
                        s0 = g * GW + hh * 512
                        nc.tensor.matmul(
                            pt[:, hh * 512 : (hh + 1) * 512],
                            w_sb,
                            src_sb[:, s0 : s0 + 512],
                            start=True,
                            stop=True,
                        )
                    nc.vector.tensor_scalar_add(
                        out=dst[:, g * GW : (g + 1) * GW], in0=pt, scalar1=bias_sb
                    )

            proj(kt_sb, wk_sb, xt_sb, bks, N, "kp")
            proj(qt_sb, wq_sb, xq_sb, bqs, HALF, "qp")
            proj(vt_sb, wv_sb, xt_sb, bvs, N, "vp")

            # ---- persistent w accumulators: chunk j -> bank j//4, partition 32*(j%4) ----
            # NOTE: no DVE memset here — a DVE write to a PSUM bank that
            # matmuls later accumulate into hangs real hardware.
            wp0 = wp.tile([128, 512], F32, tag="w0", name="wp0")
            wp1 = wp.tile([128, 512], F32, tag="w1", name="wp1")

            def emit_S(i):
                lhsT = qt_sb[:, i * 128 : (i + 1) * 128]
                tiles = []
                for g in range(NG):
                    t = sp.tile([128, GW], F32, tag="spg", name=f"sp_{i}_{g}")
                    for hh in range(2):
                        m0 = g * GW + hh * 512
                        nc.tensor.matmul(
                            t[:, hh * 512 : (hh + 1) * 512],
                            lhsT,
                            kt_sb[:, m0 : m0 + 512],
                            start=True,
                            stop=True,
                        )
                    tiles.append(t)
                return tiles

            def emit_exp(i, sptiles):
                E = epool.tile([128, N], F32, tag="E", name=f"E_{i}")
                part = small.tile([128, NG], F32, tag="part", name=f"part_{i}")
                for g in range(NG):
                    nc.scalar.activation(
                        out=E[:, g * GW : (g + 1) * GW],
                        in_=sptiles[g],
                        func=AF.Exp,
                        accum_out=part[:, g : g + 1],
                    )
                return E, part

            def emit_r(i, part):
                R = small.tile([128, 1], F32, tag="R", name=f"R_{i}")
                rr = small.tile([128, 1], F32, tag="r", name=f"r_{i}")
                nc.vector.tensor_reduce(out=R, in_=part, axis=mybir.AxisListType.X, op=ALU.add)
                nc.vector.reciprocal(out=rr, in_=R)
                return rr

            def emit_w(i, E, rr):
                for j in range(NCH):
                    bank = wp0 if j < 4 else wp1
                    jj = j % 4
                    nc.tensor.matmul(
                        bank[32 * jj : 32 * jj + 1, :],
                        rr,
                        E[:, j * 512 : (j + 1) * 512],
                        start=(i == 0),
                        stop=(i == RT - 1),
                        skip_group_check=True,
                        tile_position=(0, 32 * jj),
                    )

            # ---- main loop, software-pipelined emission ----
            exps = {}
            exps[0] = emit_exp(0, emit_S(0))
            exps[1] = emit_exp(1, emit_S(1))
            for i in range(RT):
                E, part = exps.pop(i)
                rr = emit_r(i, part)
                emit_w(i, E, rr)
                if i + 2 < RT:
                    exps[i + 2] = emit_exp(i + 2, emit_S(i + 2))

            # ---- epilogue: gather w, replicate across partitions, contract with V^T ----
            wsb = singles.tile([128, 1024], F32, tag="wsb", name="wsb")
            for j in range(NCH):
                jj = j % 4
                half = j // 4
                bank = wp0 if j < 4 else wp1
                nc.vector.tensor_copy(
                    out=wsb[32 * jj : 32 * jj + 1, half * 512 : (half + 1) * 512],
                    in_=bank[32 * jj : 32 * jj + 1, :],
                )
            opart = singles.tile([128, NCH], F32, tag="opart", name="opart")
            for j in range(NCH):
                jj = j % 4
                half = j // 4
                wrep = sp.tile([128, 512], F32, tag="spg", name=f"wrep_{j}")
                nc.tensor.matmul(
                    wrep[:, 0:512],
                    ones_sb[32 * jj : 32 * jj + 1, :],
                    wsb[32 * jj : 32 * jj + 1, half * 512 : (half + 1) * 512],
                    start=True,
                    stop=True,
                    tile_position=(32 * jj, 0),
                )
                scratch = epool.tile([128, 512], F32, tag="E", name=f"scr_{j}")
                scratch2 = small.tile([128, 512], F32, tag="scr2", name=f"scr2_{j}")
                nc.vector.tensor_mul(
                    out=scratch, in0=vt_sb[:, j * 512 : (j + 1) * 512], in1=wrep[:, 0:512]
                )
                # ACT does the free-dim sum (accum_out) while DVE moves on
                nc.scalar.activation(
                    out=scratch2,
                    in_=scratch,
                    func=AF.Identity,
                    scale=1.0 / N,
                    accum_out=opart[:, j : j + 1],
                )
            o128 = singles.tile([128, 1], F32, tag="o128", name="o128")
            nc.vector.tensor_reduce(out=o128, in_=opart, axis=mybir.AxisListType.X, op=ALU.add)
            nc.sync.dma_start(out[:, :], o128)

    nc.compile()
    return nc


_cache = {}


def get_nc():
    if "nc" not in _cache:
        _cache["nc"] = build_nc()
    return _cache["nc"]


def make_in_maps(x, Wq, bq, Wk, bk, Wv, bv):
    x = np.ascontiguousarray(np.asarray(x, np.float32))
    wqT = np.ascontiguousarray(np.asarray(Wq, np.float32).T)
    wkT = np.ascontiguousarray(np.asarray(Wk, np.float32).T)
    wvT = np.ascontiguousarray(np.asarray(Wv, np.float32).T)
    bqc = np.ascontiguousarray(np.asarray(bq, np.float32).reshape(D, 1))
    bkc = np.ascontiguousarray(np.asarray(bk, np.float32).reshape(D, 1))
    bvc = np.ascontiguousarray(np.asarray(bv, np.float32).reshape(D, 1))
    in_maps = []
    for c in range(NCORES):
        b = c // 2
        h = c % 2
        xb = x[b]
        in_maps.append(
            {
                "xt": np.ascontiguousarray(xb.T),
                "xqt": np.ascontiguousarray(xb[h * HALF : (h + 1) * HALF].T),
                "wqT": wqT,
                "wkT": wkT,
                "wvT": wvT,
                "bq": bqc,
                "bk": bkc,
                "bv": bvc,
            }
        )
    return in_maps


def combine(results):
    outs = [np.asarray(results[c]["out"]).reshape(D) for c in range(NCORES)]
    return np.stack([outs[2 * b] + outs[2 * b + 1] for b in range(B)]).astype(np.float32)


def run(inputs, trace=False, **kwargs):
    from concourse.bass_utils import run_bass_kernel_spmd

    nc = get_nc()
    in_maps = make_in_maps(**inputs)
    res = run_bass_kernel_spmd(nc, in_maps, core_ids=list(range(NCORES)), trace=trace, **kwargs)
    return combine(res.results), res


def kernel(x, Wq, bq, Wk, bk, Wv, bv):
    out, _ = run(dict(x=x, Wq=Wq, bq=bq, Wk=Wk, bk=bk, Wv=Wv, bv=bv))
    return out


# revision 13
# speedup vs baseline: 1.4131x; 1.4131x over previous
"""AttentionAggregation kernel for 8 TRN2 NeuronCores.

Math: out[b] = mean_n softmax(Q K^T)[n,:] @ V  with Q/K/V = x @ W^T + b.
Key algebraic fold: out[b,d] = sum_m w[b,m] V[b,m,d] with
  w[b,m] = (1/N) sum_n exp(S[n,m]) / R[n],  R[n] = sum_m exp(S[n,m]).
So the attn@V matmul (N^2 D work) collapses to a column-weight vector w
computed with rank-1 matmuls (r^T @ E), then a single weighted reduction
against V. Softmax max-subtraction is skipped: |S| <= ~25 here, exp stays
comfortably inside fp32 range and softmax is shift-invariant.

Precision: bf16 inputs/matmuls with fp32 PSUM accumulation end-to-end
rel err ~1.6e-3 (validated numerically against an fp64 reference).

Sharding: core c handles batch b=c//2, row half h=c%2 (2048 rows of the
4096-row softmax). Host sums the two per-core partial outputs per batch.

HW notes learned the hard way:
- fp32 matmuls lower to HI/LO pairs at half stream rate (4x slower than
  bf16 total) -> everything PE-facing is bf16.
- A DVE write (memset) to a PSUM bank that matmuls later accumulate into
  hangs the chip; PSUM zeroing must be done with a matmul (start=True).
- tensor_tensor_reduce faults on HW; use tensor_mul + activation(Identity,
  accum_out=...) instead.
"""

import sys

sys.path.insert(0, "/opt/trn_rl_repo")

import ml_dtypes
import numpy as np

import concourse.bass as bass
import concourse.mybir as mybir
import concourse.tile as tile
from concourse import bacc

D = 128
N = 4096
B = 4
NCORES = 8
HALF = N // 2  # softmax rows per core
RT = HALF // 128  # 16 row tiles per core
GW = 2048  # psum group width (4 banks) per exp instruction
NG = N // GW  # 2 exp groups per row tile
NCH = N // 512  # 8 m-chunks of 512

F32 = mybir.dt.float32
BF16 = mybir.dt.bfloat16
NPBF = ml_dtypes.bfloat16
AF = mybir.ActivationFunctionType
ALU = mybir.AluOpType


def build_nc():
    nc = bacc.Bacc()
    xt = nc.dram_tensor("xt", [D, N], BF16, kind="ExternalInput")  # x[b].T
    xqt = nc.dram_tensor("xqt", [D, HALF], BF16, kind="ExternalInput")  # row-half of x[b].T
    wqT = nc.dram_tensor("wqT", [D, D], BF16, kind="ExternalInput")  # Wq.T
    wkT = nc.dram_tensor("wkT", [D, D], BF16, kind="ExternalInput")
    wvT = nc.dram_tensor("wvT", [D, D], BF16, kind="ExternalInput")
    bq = nc.dram_tensor("bq", [D, 1], F32, kind="ExternalInput")
    bk = nc.dram_tensor("bk", [D, 1], F32, kind="ExternalInput")
    bv = nc.dram_tensor("bv", [D, 1], F32, kind="ExternalInput")
    out = nc.dram_tensor("out", [D, 1], F32, kind="ExternalOutput")

    with tile.TileContext(nc) as tc:
        with (
            tc.tile_pool(name="singles", bufs=1) as singles,
            tc.tile_pool(name="sp", bufs=2, space="PSUM") as sp,
            tc.tile_pool(name="epool", bufs=2) as epool,
        ):
            # ---- constants / weights ----
            wq_sb = singles.tile([D, D], BF16, tag="wq", name="wq_sb")
            wk_sb = singles.tile([D, D], BF16, tag="wk", name="wk_sb")
            wv_sb = singles.tile([D, D], BF16, tag="wv", name="wv_sb")
            bqs = singles.tile([D, 1], F32, tag="bq", name="bqs")
            bks = singles.tile([D, 1], F32, tag="bk", name="bks")
            bvs = singles.tile([D, 1], F32, tag="bv", name="bvs")
            ones_sb = singles.tile([D, D], BF16, tag="ones", name="ones_sb")
            nc.vector.memset(ones_sb, 1.0)
            zeros_sb = singles.tile([D, 512], BF16, tag="zeros", name="zeros_sb")
            nc.vector.memset(zeros_sb, 0.0)

            nc.sync.dma_start(wk_sb, wkT[:, :])
            nc.sync.dma_start(wq_sb, wqT[:, :])
            nc.sync.dma_start(wv_sb, wvT[:, :])

            # ---- activations in (split small so queues run in parallel) ----
            xq_sb = singles.tile([D, HALF], BF16, tag="xq", name="xq_sb")
            nc.sync.dma_start(xq_sb[:, 0:1024], xqt[:, 0:1024])
            xt_sb = singles.tile([D, N], BF16, tag="xt", name="xt_sb")
            for c in range(NCH):
                nc.sync.dma_start(xt_sb[:, c * 512 : (c + 1) * 512], xt[:, c * 512 : (c + 1) * 512])
            nc.sync.dma_start(xq_sb[:, 1024:2048], xqt[:, 1024:2048])
            nc.sync.dma_start(bks, bk[:, :])
            nc.sync.dma_start(bqs, bq[:, :])
            nc.sync.dma_start(bvs, bv[:, :])

            kt_sb = singles.tile([D, N], BF16, tag="kt", name="kt_sb")
            qt_sb = singles.tile([D, HALF], BF16, tag="qt", name="qt_sb")
            vt_sb = singles.tile([D, N], F32, tag="vt", name="vt_sb")

            # ---- projections (bf16 matmul, fp32 psum, bias add on copyback) ----
            def proj_group(dst, w_sb, src_sb, bias_sb, g, pfx):
                pt = sp.tile([128, GW], F32, tag="spg", name=f"{pfx}_{g}")
                for hh in range(2):
                    s0 = g * 1024 + hh * 512
                    nc.tensor.matmul(
                        pt[:, hh * 512 : (hh + 1) * 512],
                        w_sb,
                        src_sb[:, s0 : s0 + 512],
                        start=True,
                        stop=True,
                    )
                nc.vector.tensor_scalar_add(
                    out=dst[:, g * 1024 : (g + 1) * 1024], in0=pt[:, 0:1024], scalar1=bias_sb
                )

            def proj(dst, w_sb, src_sb, bias_sb, width, pfx):
                for g in range(width // 1024):
                    proj_group(dst, w_sb, src_sb, bias_sb, g, pfx)

            proj(kt_sb, wk_sb, xt_sb, bks, N, "kp")
            proj(qt_sb, wq_sb, xq_sb, bqs, HALF, "qp")

            # ---- per-tile scalar arrays (no pool rotation -> no slot waits) ----
            part_all = singles.tile([128, RT * NG], F32, tag="part", name="part_all")
            R_all = singles.tile([128, RT], F32, tag="R", name="R_all")
            rr_all = singles.tile([128, RT], F32, tag="rr", name="rr_all")
            rb_all = singles.tile([128, RT], BF16, tag="rb", name="rb_all")
            # fp32 SBUF accumulator for w; chunk j lives at partition 32*(j%4),
            # columns (j//4)*512.., matching the transient psum layout.
            wacc = singles.tile([128, 1024], F32, tag="wacc", name="wacc")
            nc.vector.memset(wacc, 0.0)

            def emit_S(i):
                lhsT = qt_sb[:, i * 128 : (i + 1) * 128]
                tiles = []
                for g in range(NG):
                    t = sp.tile([128, GW], F32, tag="spg", name=f"sp_{i}_{g}")
                    for hh in range(GW // 512):
                        m0 = g * GW + hh * 512
                        nc.tensor.matmul(
                            t[:, hh * 512 : (hh + 1) * 512],
                            lhsT,
                            kt_sb[:, m0 : m0 + 512],
                            start=True,
                            stop=True,
                        )
                    tiles.append(t)
                return tiles

            def emit_exp(i, sptiles):
                E = epool.tile([128, N], BF16, tag="E", name=f"E_{i}")
                for g in range(NG):
                    nc.scalar.activation(
                        out=E[:, g * GW : (g + 1) * GW],
                        in_=sptiles[g],
                        func=AF.Exp,
                        accum_out=part_all[:, NG * i + g : NG * i + g + 1],
                    )
                return E

            def emit_r(i):
                nc.vector.tensor_reduce(
                    out=R_all[:, i : i + 1],
                    in_=part_all[:, NG * i : NG * (i + 1)],
                    axis=mybir.AxisListType.X,
                    op=ALU.add,
                )
                nc.vector.reciprocal(out=rr_all[:, i : i + 1], in_=R_all[:, i : i + 1])
                nc.vector.tensor_copy(out=rb_all[:, i : i + 1], in_=rr_all[:, i : i + 1])

            def emit_w(i, E):
                rr = rb_all[:, i : i + 1]
                # transient psum: [128,1024] = 2 banks; chunk j -> bank j//4,
                # partition 32*(j%4). Zero-matmuls define every partition so the
                # DVE accumulate below reads fully-initialized PSUM.
                wt = sp.tile([128, 1024], F32, tag="spg", name=f"wt_{i}")
                for half in range(2):
                    nc.tensor.matmul(
                        wt[:, half * 512 : (half + 1) * 512],
                        zeros_sb[:, 0:128],
                        zeros_sb[:, 0:512],
                        start=True,
                        stop=False,
                        skip_group_check=True,
                    )
                for j in range(NCH):
                    jj = j % 4
                    half = j // 4
                    nc.tensor.matmul(
                        wt[32 * jj : 32 * jj + 1, half * 512 : (half + 1) * 512],
                        rr,
                        E[:, j * 512 : (j + 1) * 512],
                        start=False,
                        stop=(jj == 3),
                        skip_group_check=True,
                        tile_position=(0, 32 * jj),
                    )
                # accumulate into SBUF (frees the psum slot for the next S tile)
                for half in range(2):
                    nc.vector.tensor_add(
                        out=wacc[:, half * 512 : (half + 1) * 512],
                        in0=wacc[:, half * 512 : (half + 1) * 512],
                        in1=wt[:, half * 512 : (half + 1) * 512],
                    )

            # ---- main loop, software-pipelined emission ----
            exps = {}
            exps[0] = emit_exp(0, emit_S(0))
            exps[1] = emit_exp(1, emit_S(1))
            for i in range(RT):
                E = exps.pop(i)
                emit_r(i)
                emit_w(i, E)
                # V projection is off the critical path; one group per early
                # iteration spreads its PSUM slot pressure.
                if 2 <= i < 2 + N // 1024:
                    proj_group(vt_sb, wv_sb, xt_sb, bvs, i - 2, "vp")
                if i + 2 < RT:
                    exps[i + 2] = emit_exp(i + 2, emit_S(i + 2))

            # ---- epilogue: replicate w across partitions, contract with V^T ----
            wbb = singles.tile([128, 1024], BF16, tag="wbb", name="wbb")
            opart = singles.tile([128, NCH], F32, tag="opart", name="opart")
            nc.vector.tensor_copy(out=wbb, in_=wacc)
            for j in range(NCH):
                jj = j % 4
                half = j // 4
                cs = slice(half * 512, (half + 1) * 512)
                wrep = sp.tile([128, 512], F32, tag="spg", name=f"wrep_{j}")
                nc.tensor.matmul(
                    wrep[:, 0:512],
                    ones_sb[32 * jj : 32 * jj + 1, :],
                    wbb[32 * jj : 32 * jj + 1, cs],
                    start=True,
                    stop=True,
                    tile_position=(32 * jj, 0),
                )
                scratch = epool.tile([128, 512], F32, tag="scr", name=f"scr_{j}")
                scratch2 = epool.tile([128, 512], F32, tag="scr2", name=f"scr2_{j}")
                nc.vector.tensor_mul(
                    out=scratch, in0=vt_sb[:, j * 512 : (j + 1) * 512], in1=wrep[:, 0:512]
                )
                # ACT does the free-dim sum (accum_out) while DVE moves on
                nc.scalar.activation(
                    out=scratch2,
                    in_=scratch,
                    func=AF.Identity,
                    scale=1.0 / N,
                    accum_out=opart[:, j : j + 1],
                )
            o128 = singles.tile([128, 1], F32, tag="o128", name="o128")
            nc.vector.tensor_reduce(out=o128, in_=opart, axis=mybir.AxisListType.X, op=ALU.add)
            nc.sync.dma_start(out[:, :], o128)

    nc.compile()
    return nc


_cache = {}


def get_nc():
    if "nc" not in _cache:
        _cache["nc"] = build_nc()
    return _cache["nc"]


def make_in_maps(x, Wq, bq, Wk, bk, Wv, bv):
    x = np.asarray(x, np.float32)
    wqT = np.ascontiguousarray(np.asarray(Wq, np.float32).T.astype(NPBF))
    wkT = np.ascontiguousarray(np.asarray(Wk, np.float32).T.astype(NPBF))
    wvT = np.ascontiguousarray(np.asarray(Wv, np.float32).T.astype(NPBF))
    bqc = np.ascontiguousarray(np.asarray(bq, np.float32).reshape(D, 1))
    bkc = np.ascontiguousarray(np.asarray(bk, np.float32).reshape(D, 1))
    bvc = np.ascontiguousarray(np.asarray(bv, np.float32).reshape(D, 1))
    in_maps = []
    for c in range(NCORES):
        b = c // 2
        h = c % 2
        xbT = np.ascontiguousarray(x[b].T.astype(NPBF))  # [128, 4096] bf16
        in_maps.append(
            {
                "xt": xbT,
                "xqt": np.ascontiguousarray(xbT[:, h * HALF : (h + 1) * HALF]),
                "wqT": wqT,
                "wkT": wkT,
                "wvT": wvT,
                "bq": bqc,
                "bk": bkc,
                "bv": bvc,
            }
        )
    return in_maps


def combine(results):
    outs = [np.asarray(results[c]["out"]).reshape(D) for c in range(NCORES)]
    return np.stack([outs[2 * b] + outs[2 * b + 1] for b in range(B)]).astype(np.float32)


def run(inputs, trace=False, **kwargs):
    from concourse.bass_utils import run_bass_kernel_spmd

    nc = get_nc()
    in_maps = make_in_maps(**inputs)
    res = run_bass_kernel_spmd(nc, in_maps, core_ids=list(range(NCORES)), trace=trace, **kwargs)
    return combine(res.results), res


def kernel(x, Wq, bq, Wk, bk, Wv, bv):
    out, _ = run(dict(x=x, Wq=Wq, bq=bq, Wk=Wk, bk=bk, Wv=Wv, bv=bv))
    return out
